# revision 7
# baseline (speedup 1.0000x reference)
"""BinaryDense forward on 8 Trainium2 NeuronCores.

Computes out = x @ (w_raw > 0) for x[4096,4096] f32, w_raw[4096,4096] f32.
(The straight-through-estimator forward is exactly the hard binary matmul.)

Sharding: 4 batch groups x 2 feature groups (one core each).
Per core: out.T[Nc=2048, Mc=1024] = (w_bin shard).T @ (x shard), K=4096.

Default device kernel ("fast", single-pass bf16, PE-roofline-bound):
  - x.T shard streamed in 128-row k-chunks, cast f32->bf16 on DVE and
    cached in SBUF (binarized w in bf16 {0,1} is exact; x's bf16
    rounding over ~50-term output sums gives ~1.7e-3 l2 rel err),
  - w streamed per (256-col n-tile, 4 k-chunks) as one packed
    [128,4,256] DMA, binarized to bf16 {0,1} on DVE,
  - matmuls with w slices stationary accumulate K/128 chunks into 4
    PSUM banks per n-tile, banks double-buffered across n-tiles so
    ScalarE eviction overlaps the next n-tile's accumulation,
  - in repeated (timing) programs, the next iteration's x-prep is
    interleaved into the current n-tile loop with iteration-parity
    SBUF tags so the PE never stalls at iteration boundaries.

Legacy modes (split/splith/bf16/f32r/drsplit) kept for comparison.
"""

import numpy as np

_NCORES = 8
_G1 = 4  # batch groups
_G2 = 2  # feature groups
_B = 4096
_D = 4096
_F = 4096

_cache = {}
_DR_SCALE = 512.0


def _build(K, Mc, Nc, mode="split", repeat=1, opts=None):
    """Build + compile the per-core Bass program.

    xt: [K, Mc] f32 (x shard, transposed), wr: [K, Nc] f32 (w_raw shard),
    outT: [Nc, Mc] f32.
    """
    import concourse.bacc as bacc
    import concourse.tile as tile
    from concourse import mybir

    opts = dict(opts or {})
    wbufs = opts.get("wbufs", 4)
    evbufs = opts.get("evbufs", 4)
    xsbufs = opts.get("xsbufs", 3)
    ev_eng = opts.get("ev_eng", "vector")

    if mode == "fast":
        return _build_fast(K, Mc, Nc, repeat, opts)
    if mode.startswith("fp8c"):
        opts = dict(opts)
        if mode == "fp8cl":
            opts["ldw"] = True
        return _build_fp8c(K, Mc, Nc, repeat, opts)
    if mode.startswith("mix8"):
        # mix8[_<k8>]: first k8 of K contracted in fp8e4 DoubleRow pairs
        opts = dict(opts)
        opts["k8"] = int(mode.split("_")[1]) if "_" in mode else 1024
        return _build_fast(K, Mc, Nc, repeat, opts)

    dt = mybir.dt
    P = 128
    NT = 512  # n-tile (psum free dim)
    MC = 512  # m moving chunk
    KC = K // P
    NTC = Nc // NT
    MCC = Mc // MC
    NNC = NT // P  # stationary 128-slices per n-tile

    nc = bacc.Bacc(None, target_bir_lowering=False, debug=False, num_devices=_NCORES)

    xt_d = nc.dram_tensor("xt", [K, Mc], dt.float32, kind="ExternalInput")
    wr_d = nc.dram_tensor("wr", [K, Nc], dt.float32, kind="ExternalInput")
    outT_d = nc.dram_tensor("outT", [Nc, Mc], dt.float32, kind="ExternalOutput")

    with tile.TileContext(nc) as tc:
        with (
            tc.tile_pool(name="xcache", bufs=1) as xcache,
            tc.tile_pool(name="xstage", bufs=xsbufs) as xstage,
            tc.tile_pool(name="wpool", bufs=wbufs) as wpool,
            tc.tile_pool(name="evpool", bufs=evbufs) as evpool,
            tc.tile_pool(name="pspool", bufs=1, space="PSUM") as pspool,
        ):
            def emit_xprep(k, xparts, xdrs):
                # x chunk k: DMA f32 -> split to bf16 hi + lo (or cast)
                xs = xstage.tile([P, Mc], dt.float32, name="xs", tag="xs")
                nc.sync.dma_start(xs[:], xt_d[k * P : (k + 1) * P, :])
                if mode in ("split", "splith"):
                    hdt = dt.bfloat16 if mode == "split" else dt.float16
                    xhi = xcache.tile([P, Mc], hdt, name=f"xhi{k}", tag=f"xhi{k}")
                    nc.scalar.copy(xhi[:], xs[:])
                    xlo = xcache.tile([P, Mc], hdt, name=f"xlo{k}", tag=f"xlo{k}")
                    nc.vector.tensor_sub(xlo[:], xs[:], xhi[:])
                    xparts.append((xhi, xlo))
                elif mode == "drsplit":
                    xhi = xcache.tile([P, Mc], dt.bfloat16, name=f"xhi{k}", tag=f"xhi{k}")
                    nc.scalar.copy(xhi[:], xs[:])
                    xlo = xstage.tile([P, Mc], dt.bfloat16, name="xlo", tag="xlo")
                    nc.vector.tensor_sub(xlo[:], xs[:], xhi[:])
                    # pack scaled fp8 residual into DR plane k%2 of chunk k//2
                    xparts.append(xhi)
                    k2 = k // 2
                    if k % 2 == 0:
                        xdr = xcache.tile(
                            [P, 2, Mc], dt.float8e4, name=f"xdr{k2}", tag=f"xdr{k2}"
                        )
                        xdrs.append(xdr)
                    nc.vector.tensor_scalar_mul(
                        xdrs[k2][:, k % 2, :], xlo[:], _DR_SCALE
                    )
                elif mode == "f32r":
                    xr = xcache.tile([P, Mc], dt.float32r, name=f"xr{k}", tag=f"xr{k}")
                    nc.vector.tensor_copy(xr[:], xs[:])
                    xparts.append((xr,))
                elif mode == "bf16":
                    xhi = xcache.tile([P, Mc], dt.bfloat16, name=f"xhi{k}", tag=f"xhi{k}")
                    nc.scalar.copy(xhi[:], xs[:])
                    xparts.append((xhi,))
                else:
                    raise ValueError(mode)

            def emit_body():
                xparts = []
                xdrs = []
                for k in range(KC):
                    emit_xprep(k, xparts, xdrs)
                wdt = {
                    "split": dt.bfloat16,
                    "splith": dt.float16,
                    "bf16": dt.bfloat16,
                    "f32r": dt.float32r,
                    "drsplit": dt.bfloat16,
                }[mode]

                if mode == "drsplit":
                    for nt in range(NTC):
                        psums = {}
                        for nn in range(NNC):
                            for mc in range(MCC):
                                psums[(nn, mc)] = pspool.tile(
                                    [P, MC], dt.float32,
                                    name=f"ps{nn}_{mc}", tag=f"ps{nn}_{mc}",
                                )
                        wdrs = {}
                        # hi pass: bf16, K chunks of 128
                        for k in range(KC):
                            wf = wpool.tile([P, NT], dt.float32, name="wf", tag="wf")
                            nc.sync.dma_start(
                                wf[:], wr_d[k * P : (k + 1) * P, nt * NT : (nt + 1) * NT]
                            )
                            wb = wpool.tile([P, NT], dt.bfloat16, name="wb", tag="wb")
                            nc.vector.tensor_scalar(
                                wb[:], wf[:], 0.0, None, mybir.AluOpType.is_gt
                            )
                            # also binarize into the fp8 DR plane for the lo pass
                            k2, kp = k // 2, k % 2
                            if kp == 0:
                                wdrs[k2] = wpool.tile(
                                    [P, 2, NT], dt.float8e4,
                                    name="wdr", tag="wdr", bufs=KC // 2 + 2,
                                )
                            nc.vector.tensor_scalar(
                                wdrs[k2][:, kp, :], wf[:], 0.0, None,
                                mybir.AluOpType.is_gt,
                            )
                            for nn in range(NNC):
                                for mc in range(MCC):
                                    nc.tensor.matmul(
                                        psums[(nn, mc)][:],
                                        wb[:, nn * P : (nn + 1) * P],
                                        xparts[k][:, mc * MC : (mc + 1) * MC],
                                        start=(k == 0),
                                        stop=(k == KC - 1),
                                    )
                        # evict hi results to SBUF, then lo pass reuses banks
                        hiparts = {}
                        for nn in range(NNC):
                            for mc in range(MCC):
                                hv = evpool.tile(
                                    [P, MC], dt.float32,
                                    name=f"hi{nn}_{mc}", tag=f"hi{nn}_{mc}", bufs=2,
                                )
                                nc.scalar.copy(hv[:], psums[(nn, mc)][:])
                                hiparts[(nn, mc)] = hv
                        psums2 = {}
                        for nn in range(NNC):
                            for mc in range(MCC):
                                psums2[(nn, mc)] = pspool.tile(
                                    [P, MC], dt.float32,
                                    name=f"ps{nn}_{mc}", tag=f"ps{nn}_{mc}",
                                )
                        for k2 in range(KC // 2):
                            for nn in range(NNC):
                                for mc in range(MCC):
                                    nc.tensor.matmul(
                                        psums2[(nn, mc)][:],
                                        wdrs[k2][:, :, nn * P : (nn + 1) * P],
                                        xdrs[k2][:, :, mc * MC : (mc + 1) * MC],
                                        start=(k2 == 0),
                                        stop=(k2 == KC // 2 - 1),
                                        perf_mode=mybir.MatmulPerfMode.DoubleRow,
                                    )
                        for nn in range(NNC):
                            for mc in range(MCC):
                                ev = evpool.tile([P, MC], dt.float32, name="ev", tag="ev")
                                nc.vector.scalar_tensor_tensor(
                                    ev[:], psums2[(nn, mc)][:], 1.0 / _DR_SCALE,
                                    hiparts[(nn, mc)][:],
                                    mybir.AluOpType.mult, mybir.AluOpType.add,
                                )
                                nc.sync.dma_start(
                                    outT_d[
                                        nt * NT + nn * P : nt * NT + (nn + 1) * P,
                                        mc * MC : (mc + 1) * MC,
                                    ],
                                    ev[:],
                                )
                    return

                # ---- main: per n-tile, accumulate over k into 8 psum banks ----
                for nt in range(NTC):
                    psums = {}
                    for nn in range(NNC):
                        for mc in range(MCC):
                            psums[(nn, mc)] = pspool.tile(
                                [P, MC],
                                dt.float32,
                                name=f"ps{nn}_{mc}",
                                tag=f"ps{nn}_{mc}",
                            )
                    nparts = len(xparts[0])
                    for k in range(KC):
                        wf = wpool.tile([P, NT], dt.float32, name="wf", tag="wf")
                        nc.sync.dma_start(
                            wf[:], wr_d[k * P : (k + 1) * P, nt * NT : (nt + 1) * NT]
                        )
                        wb = wpool.tile([P, NT], wdt, name="wb", tag="wb")
                        nc.vector.tensor_scalar(
                            wb[:], wf[:], 0.0, None, mybir.AluOpType.is_gt
                        )
                        for nn in range(NNC):
                            for pi in range(nparts):
                                for mc in range(MCC):
                                    nc.tensor.matmul(
                                        psums[(nn, mc)][:],
                                        wb[:, nn * P : (nn + 1) * P],
                                        xparts[k][pi][:, mc * MC : (mc + 1) * MC],
                                        start=(k == 0 and pi == 0),
                                        stop=(k == KC - 1 and pi == nparts - 1),
                                    )
                    for nn in range(NNC):
                        for mc in range(MCC):
                            ev = evpool.tile([P, MC], dt.float32, name="ev", tag="ev")
                            if ev_eng == "vector":
                                nc.vector.tensor_copy(ev[:], psums[(nn, mc)][:])
                            else:
                                nc.scalar.copy(ev[:], psums[(nn, mc)][:])
                            nc.sync.dma_start(
                                outT_d[
                                    nt * NT + nn * P : nt * NT + (nn + 1) * P,
                                    mc * MC : (mc + 1) * MC,
                                ],
                                ev[:],
                            )

            if repeat == 1:
                emit_body()
            elif opts.get("unroll"):
                for _ in range(repeat):
                    emit_body()
            else:
                with tc.For_i(0, repeat, 1):
                    emit_body()

    nc.compile()
    return nc


def _build_fp8c(K, Mc, Nc, repeat=1, opts=None):
    """Error-compensated fp8 DoubleRow kernel; all input prep on host.

    Host ships (per core):
      xhi [P, KP, 2, Mc] fp8e4 = fp8(xt), xlo = fp8(xt - fp8(xt)),
      w8  [NTC*P, KP, 2, NT] fp8e4 = binarized w in DR pair layout.
    Contraction maps k -> (kp, j, p) consistently on both operands, so any
    layout permutation is valid. Each DR matmul contracts 256 k-values at
    0.5 cyc/row; hi and lo passes share the same stationary w tile and
    accumulate into the same PSUM bank, so compensation costs no extra
    weight traffic. PE work: NTC*KP*NNC*4 matmuls of FD=MC.
    """
    import concourse.bacc as bacc
    import concourse.tile as tile
    from concourse import mybir

    opts = dict(opts or {})
    wbufs = opts.get("wbufs", 3)
    evbufs = opts.get("evbufs", 4)
    NT = opts.get("nt", 256)  # n-tile width (psum group)
    MC = opts.get("mc", 512)  # psum free dim (one bank)
    ldw = opts.get("ldw", False)  # explicit ldweights, non-self-loading matmuls

    dt = mybir.dt
    P = 128
    KP = K // 256  # DR pairs (256 k-values each)
    NTC = Nc // NT
    NNC = NT // P
    MCC = Mc // MC

    nc = bacc.Bacc(None, target_bir_lowering=False, debug=False, num_devices=_NCORES)

    xhi_d = nc.dram_tensor("xhi", [P, KP, 2, Mc], dt.float8e4, kind="ExternalInput")
    xlo_d = nc.dram_tensor("xlo", [P, KP, 2, Mc], dt.float8e4, kind="ExternalInput")
    w8_d = nc.dram_tensor("w8", [NTC * P, KP, 2, NT], dt.float8e4, kind="ExternalInput")
    outT_d = nc.dram_tensor("outT", [Nc, Mc], dt.float32, kind="ExternalOutput")

    with tile.TileContext(nc) as tc:
        with (
            tc.tile_pool(name="xpool", bufs=1) as xpool,
            tc.tile_pool(name="wpool", bufs=wbufs) as wpool,
            tc.tile_pool(name="evpool", bufs=evbufs) as evpool,
            tc.tile_pool(name="pspool", bufs=1, space="PSUM") as pspool,
        ):
            xtiles = {}

            def emit_xload(it):
                par = it % 2
                xhi = xpool.tile(
                    [P, KP, 2, Mc], dt.float8e4, name="xhi", tag=f"xhi_{par}"
                )
                nc.sync.dma_start(xhi[:], xhi_d[:])
                xlo = xpool.tile(
                    [P, KP, 2, Mc], dt.float8e4, name="xlo", tag=f"xlo_{par}"
                )
                nc.sync.dma_start(xlo[:], xlo_d[:])
                xtiles[par] = (xhi, xlo)

            def emit_main(it, prep_next):
                par = it % 2
                xhi, xlo = xtiles[par]
                for nt in range(NTC):
                    ps = {}
                    for nn in range(NNC):
                        for mc in range(MCC):
                            ps[(nn, mc)] = pspool.tile(
                                [P, MC],
                                dt.float32,
                                name=f"ps{nn}_{mc}",
                                tag=f"ps{nn}_{mc}_{nt % 2}",
                            )
                    wt = wpool.tile([P, KP, 2, NT], dt.float8e4, name="wt", tag="wt")
                    nc.sync.dma_start(wt[:], w8_d[nt * P : (nt + 1) * P])
                    if prep_next and nt == 1:
                        emit_xload(it + 1)
                    for kp in range(KP):
                        for nn in range(NNC):
                            wsl = wt[:, kp, :, nn * P : (nn + 1) * P]
                            if ldw:
                                nc.tensor.ldweights(
                                    wsl, perf_mode=mybir.MatmulPerfMode.DoubleRow
                                )
                            for xi, xsrc in enumerate((xhi, xlo)):
                                for mc in range(MCC):
                                    mm = nc.tensor.matmul(
                                        ps[(nn, mc)][:],
                                        wsl,
                                        xsrc[:, kp, :, mc * MC : (mc + 1) * MC],
                                        start=(kp == 0 and xi == 0),
                                        stop=(kp == KP - 1 and xi == 1),
                                        perf_mode=mybir.MatmulPerfMode.DoubleRow,
                                    )
                                    if ldw:
                                        mm.ins.ldweights = False
                    for nn in range(NNC):
                        for mc in range(MCC):
                            ev = evpool.tile([P, MC], dt.float32, name="ev", tag="ev")
                            nc.scalar.copy(ev[:], ps[(nn, mc)][:])
                            nc.sync.dma_start(
                                outT_d[
                                    nt * NT + nn * P : nt * NT + (nn + 1) * P,
                                    mc * MC : (mc + 1) * MC,
                                ],
                                ev[:],
                            )

            emit_xload(0)
            for it in range(repeat):
                emit_main(it, prep_next=(it + 1 < repeat))

    nc.compile()
    return nc


def _build_fast(K, Mc, Nc, repeat=1, opts=None):
    """Single-pass bf16 kernel, pipelined.

    out.T[Nc, Mc] = binarize(wr).T @ x, K contraction. Per iteration:
      - x.T shard streamed in 128-row chunks, cast f32->bf16, cached in
        SBUF with iteration-parity tags (next iteration's x-prep overlaps
        this iteration's matmuls; prep is hand-interleaved into the n-tile
        loop so the DVE FIFO never delays binarize).
      - w streamed per (n-tile of 256, group of 4 k-chunks) as one packed
        [128, 4, 256] DMA, binarized to bf16 {0,1} on DVE.
      - matmuls accumulate 32 k-chunks into 4 PSUM banks per n-tile;
        banks double-buffered across n-tiles (nt parity) so eviction
        (ScalarE -> SBUF -> DMA) overlaps the next n-tile's accumulation.
    """
    import concourse.bacc as bacc
    import concourse.tile as tile
    from concourse import mybir

    opts = dict(opts or {})
    wbufs = opts.get("wbufs", 4)
    evbufs = opts.get("evbufs", 4)
    xsbufs = opts.get("xsbufs", 3)
    KJ = opts.get("kj", 4)  # k-chunks per w DMA
    NT = opts.get("nt", 256)  # n-tile (psum group width)
    K8 = opts.get("k8", 0)  # leading k-values contracted in fp8 DoubleRow
    assert K8 % 256 == 0

    dt = mybir.dt
    P = 128
    MC = 512  # psum free dim (one bank)
    KC = K // P  # 32 k-chunks
    KP = K8 // 256  # fp8 DoubleRow pairs (2 k-chunks each)
    KB0 = 2 * KP  # first bf16 k-chunk
    KG = (KC - KB0) // KJ  # bf16 w DMA groups per n-tile
    assert (KC - KB0) % KJ == 0
    NTC = Nc // NT  # 8 n-tiles
    NNC = NT // P  # 2 stationary slices per n-tile
    MCC = Mc // MC  # 2 moving chunks

    nc = bacc.Bacc(None, target_bir_lowering=False, debug=False, num_devices=_NCORES)

    xt_d = nc.dram_tensor("xt", [K, Mc], dt.float32, kind="ExternalInput")
    wr_d = nc.dram_tensor("wr", [K, Nc], dt.float32, kind="ExternalInput")
    outT_d = nc.dram_tensor("outT", [Nc, Mc], dt.float32, kind="ExternalOutput")

    with tile.TileContext(nc) as tc:
        with (
            tc.tile_pool(name="xcache", bufs=1) as xcache,
            tc.tile_pool(name="xstage", bufs=xsbufs) as xstage,
            tc.tile_pool(name="wpool", bufs=wbufs) as wpool,
            tc.tile_pool(name="evpool", bufs=evbufs) as evpool,
            tc.tile_pool(name="pspool", bufs=1, space="PSUM") as pspool,
        ):
            xbs = {}  # parity -> {'8': [pair tiles], 'b': {k: bf16 tile}}
            # x-prep work items, in consumption order
            prep_items = [("8", kp) for kp in range(KP)] + [
                ("b", k) for k in range(KB0, KC)
            ]

            def emit_xprep_item(it, item):
                par = it % 2
                if par not in xbs or item == prep_items[0]:
                    xbs[par] = {"8": [], "b": {}}
                kind, idx = item
                if kind == "8":
                    xs8 = xstage.tile(
                        [P, 2, Mc], dt.float32, name="xs8", tag="xs8", bufs=2
                    )
                    src = xt_d[idx * 2 * P : (idx + 1) * 2 * P, :].rearrange(
                        "(j p) m -> p j m", p=P
                    )
                    nc.sync.dma_start(xs8[:], src)
                    x8 = xcache.tile(
                        [P, 2, Mc], dt.float8e4, name=f"x8{idx}", tag=f"x8{idx}_{par}"
                    )
                    nc.vector.tensor_copy(x8[:], xs8[:])
                    xbs[par]["8"].append(x8)
                else:
                    k = idx
                    xs = xstage.tile([P, Mc], dt.float32, name="xs", tag="xs")
                    nc.sync.dma_start(xs[:], xt_d[k * P : (k + 1) * P, :])
                    xb = xcache.tile(
                        [P, Mc], dt.bfloat16, name=f"xb{k}", tag=f"xb{k}_{par}"
                    )
                    nc.vector.tensor_copy(xb[:], xs[:])
                    xbs[par]["b"][k] = xb

            def emit_main(it, prep_next):
                par = it % 2
                xc = xbs[par]
                for nt in range(NTC):
                    ps = {}
                    for nn in range(NNC):
                        for mc in range(MCC):
                            ps[(nn, mc)] = pspool.tile(
                                [P, MC],
                                dt.float32,
                                name=f"ps{nn}_{mc}",
                                tag=f"ps{nn}_{mc}_{nt % 2}",
                            )
                    # fp8 DoubleRow pairs over k[0 : K8)
                    for kp in range(KP):
                        wf8 = wpool.tile(
                            [P, 2, NT], dt.float32, name="wf8", tag="wf8", bufs=wbufs
                        )
                        src = wr_d[
                            kp * 2 * P : (kp + 1) * 2 * P, nt * NT : (nt + 1) * NT
                        ].rearrange("(j p) n -> p j n", p=P)
                        nc.sync.dma_start(wf8[:], src)
                        wb8 = wpool.tile(
                            [P, 2, NT], dt.float8e4, name="wb8", tag="wb8", bufs=wbufs
                        )
                        nc.vector.tensor_scalar(
                            wb8[:], wf8[:], 0.0, None, mybir.AluOpType.is_gt
                        )
                        for nn in range(NNC):
                            for mc in range(MCC):
                                nc.tensor.matmul(
                                    ps[(nn, mc)][:],
                                    wb8[:, :, nn * P : (nn + 1) * P],
                                    xc["8"][kp][:, :, mc * MC : (mc + 1) * MC],
                                    start=(kp == 0),
                                    stop=(kp == KP - 1 and KG == 0),
                                    perf_mode=mybir.MatmulPerfMode.DoubleRow,
                                )
                    # bf16 over k[K8 : K)
                    for kg in range(KG):
                        wf = wpool.tile([P, KJ, NT], dt.float32, name="wf", tag="wf")
                        c0 = KB0 + kg * KJ
                        src = wr_d[
                            c0 * P : (c0 + KJ) * P, nt * NT : (nt + 1) * NT
                        ].rearrange("(j p) n -> p j n", p=P)
                        nc.sync.dma_start(wf[:], src)
                        wb = wpool.tile([P, KJ, NT], dt.bfloat16, name="wb", tag="wb")
                        nc.vector.tensor_scalar(
                            wb[:], wf[:], 0.0, None, mybir.AluOpType.is_gt
                        )
                        for j in range(KJ):
                            k = c0 + j
                            for nn in range(NNC):
                                for mc in range(MCC):
                                    nc.tensor.matmul(
                                        ps[(nn, mc)][:],
                                        wb[:, j, nn * P : (nn + 1) * P],
                                        xc["b"][k][:, mc * MC : (mc + 1) * MC],
                                        start=(k == 0 and KP == 0),
                                        stop=(k == KC - 1),
                                    )
                    # interleave next iteration's x-prep across n-tiles
                    if prep_next:
                        n_items = len(prep_items)
                        lo = nt * n_items // NTC
                        hi = (nt + 1) * n_items // NTC
                        for item in prep_items[lo:hi]:
                            emit_xprep_item(it + 1, item)
                    for nn in range(NNC):
                        for mc in range(MCC):
                            ev = evpool.tile([P, MC], dt.float32, name="ev", tag="ev")
                            nc.scalar.copy(ev[:], ps[(nn, mc)][:])
                            nc.sync.dma_start(
                                outT_d[
                                    nt * NT + nn * P : nt * NT + (nn + 1) * P,
                                    mc * MC : (mc + 1) * MC,
                                ],
                                ev[:],
                            )

            for item in prep_items:
                emit_xprep_item(0, item)
            for it in range(repeat):
                emit_main(it, prep_next=(it + 1 < repeat))

    nc.compile()
    return nc


def _get_nc(K, Mc, Nc, mode="split", repeat=1, opts=None):
    key = (K, Mc, Nc, mode, repeat, tuple(sorted((opts or {}).items())))
    if key not in _cache:
        _cache[key] = _build(K, Mc, Nc, mode, repeat, opts)
    return _cache[key]


def prepare_in_maps(x, w_raw, mode):
    """Per-core input dicts for the given mode (host-side shard + prep)."""
    B, D = x.shape
    F = w_raw.shape[1]
    Mc = B // _G1
    Nc = F // _G2
    xt = np.ascontiguousarray(x.T)  # [D, B]

    if mode.startswith("fp8c"):
        from concourse import mybir

        f8 = mybir.dt.np(mybir.dt.float8e4)
        P = 128
        KP = D // 256
        NT = 256
        NTC = Nc // NT
        in_maps = []
        for c in range(_NCORES):
            i, j = c // _G2, c % _G2
            xt_c = xt[:, i * Mc : (i + 1) * Mc]  # [D, Mc]
            # k -> (kp, plane, partition); same mapping used for w below.
            xr = np.ascontiguousarray(
                xt_c.reshape(KP, 2, P, Mc).transpose(2, 0, 1, 3)
            )  # [P, KP, 2, Mc]
            xhi = xr.astype(f8)
            xlo = (xr - xhi.astype(np.float32)).astype(f8)
            wr_c = w_raw[:, j * Nc : (j + 1) * Nc]  # [D, Nc]
            wb = (wr_c > 0).astype(f8).reshape(KP, 2, P, NTC, NT)
            w8 = np.ascontiguousarray(wb.transpose(3, 2, 0, 1, 4)).reshape(
                NTC * P, KP, 2, NT
            )
            in_maps.append(
                {
                    "xhi": np.ascontiguousarray(xhi),
                    "xlo": np.ascontiguousarray(xlo),
                    "w8": w8,
                }
            )
        return in_maps

    in_maps = []
    for c in range(_NCORES):
        i, j = c // _G2, c % _G2
        in_maps.append(
            {
                "xt": np.ascontiguousarray(xt[:, i * Mc : (i + 1) * Mc]),
                "wr": np.ascontiguousarray(w_raw[:, j * Nc : (j + 1) * Nc]),
            }
        )
    return in_maps


def _run(x, w_raw, mode="split", repeat=1):
    """Shard, run on 8 cores, gather. x:[B,D] f32, w_raw:[D,F] f32."""
    from concourse.bass_utils import run_bass_kernel_spmd

    B, D = x.shape
    D2, F = w_raw.shape
    assert D == D2
    Mc = B // _G1
    Nc = F // _G2

    nc = _get_nc(D, Mc, Nc, mode, repeat)

    in_maps = prepare_in_maps(x, w_raw, mode)
    res = run_bass_kernel_spmd(nc, in_maps, list(range(_NCORES)))

    outT = np.empty((F, B), dtype=np.float32)
    for c in range(_NCORES):
        i, j = c // _G2, c % _G2
        outT[j * Nc : (j + 1) * Nc, i * Mc : (i + 1) * Mc] = res.results[c]["outT"]
    return np.ascontiguousarray(outT.T)


def kernel(x, w_raw):
    x = np.asarray(x, dtype=np.float32)
    w_raw = np.asarray(w_raw, dtype=np.float32)
    # The device occasionally throws a transient NRT_EXEC_UNIT_UNRECOVERABLE;
    # a fresh attempt (after clearing jax backends) usually succeeds.
    last = None
    for attempt in range(3):
        try:
            return _run(x, w_raw, mode="fast", repeat=1)
        except Exception as e:  # noqa: BLE001 - retry transient device faults
            last = e
            import time as _time

            _time.sleep(5)
            try:
                import jax

                jax.clear_caches()
                jax._src.api.clear_backends()
            except Exception:
                pass
    raise last

